# revision 1
# baseline (speedup 1.0000x reference)
"""Trainium2 Bass kernel for nn_AdaptiveBoundaryRefinement_45861660787095.

Self-contained: takes FULL inputs (B=16,M=128,T=12000), shards batch across 8
NeuronCores (2 samples/core), runs a Bass/Tile kernel per core, gathers.

Algorithm notes (validated vs reference by numpy prototype + CoreSim):
- The reference's batch-global early-stop is a mathematical no-op: `done` only
  becomes True when adj==0 everywhere, which is already a fixed point (cons and
  the local means are iteration-invariant).
- The 5 refinement iterations collapse to a closed form (the per-step clip
  never changes active-set decisions), computed with predicated copies.
- Layout: per sample, time is chunked into 47 chunks of 256 (12032 padded);
  chunk-layout rows = sample*47+chunk on partitions, 256 steps on free dim.
- Streaming: mel tiles [128, ~2054]; ScalarE squares (bf16 out), VectorE
  cross-products (bf16 out), PE one-hot-column reductions (f32r for S, bf16
  for Q/D; ~260-col overlapped windows) write S/Q/D directly into the chunk
  layout INCLUDING halo columns, so no cross-partition copies are needed.
- Compute instructions require SBUF start partition in {0,32,64,96}; all
  per-row edge fixes use inline-constant masks (DMA is exempt).
- temporal enters only through per-sample scalar thresholds (0.7/0.4 - w2*t),
  so the temporal chain runs fully parallel to the cons chain.
"""

import sys

import numpy as np

_TRN_REPO = "/opt/trn_rl_repo"
if _TRN_REPO not in sys.path:
    sys.path.insert(0, _TRN_REPO)

import concourse.bass as bass
import concourse.bacc as bacc
import concourse.mybir as mybir
import concourse.tile as tile
from concourse.bass_utils import run_bass_kernel_spmd

F32 = mybir.dt.float32
F32R = mybir.dt.float32r
BF16 = mybir.dt.bfloat16
ALU = mybir.AluOpType
ACTF = mybir.ActivationFunctionType
AX = mybir.AxisListType

B, M, T = 16, 128, 12000
NCORES = 8
BPC = B // NCORES            # samples per core = 2
CH = 256                     # chunk width
NCH = (T + CH - 1) // CH     # 47 chunks per sample
ROWS = BPC * NCH             # 94 chunk-rows
EPS = 1e-8
GRAD_THRESH = 0.15
LASTW = T - CH * (NCH - 1)   # 224 real cols in the last chunk
NGS = [1, 8, 8, 8, 8, 8, 6]  # chunks per mel tile (small first tile)


def _softmax_f32(x):
    x = np.asarray(x, np.float32)
    m = np.max(x).astype(np.float32)
    e = np.exp((x - m).astype(np.float32)).astype(np.float32)
    return (e / e.sum(dtype=np.float32).astype(np.float32)).astype(np.float32)


def _const_masks():
    # validC [ROWS, 260]: 1 where t in [0, T), else 0 (t = 256p - 2 + h)
    validC = np.ones((ROWS, 260), np.float32)
    for r0 in (0, NCH):
        validC[r0, 0:2] = 0.0
    for r0 in (NCH - 1, ROWS - 1):
        validC[r0, LASTW + 2 : 260] = 0.0  # h >= 226 -> t >= 12000
    # validD [ROWS, 256]: ddir gate * 0.1: 0 at t=0, t >= T-1, and pad
    validD = np.full((ROWS, 256), np.float32(0.1), np.float32)
    for r0 in (0, NCH):
        validD[r0, 0] = 0.0
    for r0 in (NCH - 1, ROWS - 1):
        validD[r0, LASTW - 1 : 256] = 0.0  # col 223.. -> t >= 11999
    # scalemask [ROWS, 256]: local = LS * scalemask; 0.25 at t=1 and t=T-2
    scalemask = np.full((ROWS, 256), np.float32(0.2), np.float32)
    for r0 in (0, NCH):
        scalemask[r0, 1] = 0.25
    for r0 in (NCH - 1, ROWS - 1):
        scalemask[r0, LASTW - 2] = 0.25  # col 222 -> t = 11998
    # ind2 [2, ROWS]: broadcast lhsT (sample s -> its rows)
    ind2 = np.zeros((2, ROWS), np.float32)
    ind2[0, 0:NCH] = 1.0
    ind2[1, NCH:ROWS] = 1.0
    return validC, validD, scalemask, ind2


def build_nc(w0, w1, w2):
    import ml_dtypes

    nc = bacc.Bacc("TRN2", target_bir_lowering=False, debug=False)
    mel = nc.dram_tensor("mel_features", [BPC, M, T], F32R, kind="ExternalInput")
    spec = nc.dram_tensor("spectral_features", [BPC, T], F32, kind="ExternalInput")
    init = nc.dram_tensor("initial_boundaries", [BPC, T], F32, kind="ExternalInput")
    out = nc.dram_tensor("out", [BPC, T], F32, kind="ExternalOutput")

    validC_np, validD_np, scalemask_np, ind2_np = _const_masks()
    wz_np = np.zeros((128, 257), np.float32)
    wz_np[:, 128] = 1.0
    wzb_np = np.zeros((128, 257), ml_dtypes.bfloat16)
    wzb_np[:, 128] = 1.0
    zeros_np = np.zeros((128, 291), np.float32)
    validC_d = nc.inline_tensor(validC_np, name="validC")
    validD_d = nc.inline_tensor(validD_np, name="validD")
    scalemask_d = nc.inline_tensor(scalemask_np, name="scalemask")
    ind2_d = nc.inline_tensor(ind2_np, name="ind2c")
    wz_d = nc.inline_tensor(wz_np, name="wzc")
    wzb_d = nc.inline_tensor(wzb_np, name="wzbc")
    zeros_d = nc.inline_tensor(zeros_np, name="zeroc")

    with tile.TileContext(nc) as tc:
        with (
            tc.tile_pool(name="mel", bufs=3) as pmel,
            tc.tile_pool(name="sq", bufs=2) as psq,
            tc.tile_pool(name="cross", bufs=2) as pcross,
            tc.tile_pool(name="stat", bufs=1) as pstat,
            tc.tile_pool(name="ps", bufs=1, space="PSUM") as pps,
            tc.tile_pool(name="ps2", bufs=1, space="PSUM") as pps2,
        ):
            # ---------------- constants / persistent tiles ----------------
            # WZ(b): zeros with a ones-column at col 128; slice
            # [:, 128-r : 256-r] routes a ones-reduction into out row r.
            WZ = pstat.tile([128, 257], F32R)
            nc.gpsimd.dma_start(out=WZ, in_=wz_d[:, :])
            WZb = pstat.tile([128, 257], BF16)
            nc.gpsimd.dma_start(out=WZb, in_=wzb_d[:, :])
            # persistent first/last mel tiles (per sample), pads zeroed once
            W_FIRST = NGS[0] * CH + 6
            W_LAST = NGS[-1] * CH + 6
            T0_LAST = (NCH - NGS[-1]) * CH
            LASTREAL = T - (T0_LAST - 3)
            melt_firsts, melt_lasts = [], []
            for bb in range(BPC):
                mf = pstat.tile([128, W_FIRST], F32R, tag=f"mf{bb}")
                nc.gpsimd.dma_start(out=mf[:, 0:3], in_=zeros_d[:, 0:3])
                melt_firsts.append(mf)
                ml = pstat.tile([128, W_LAST], F32R, tag=f"ml{bb}")
                nc.gpsimd.dma_start(
                    out=ml[:, LASTREAL:W_LAST],
                    in_=zeros_d[:, 0 : W_LAST - LASTREAL],
                )
                melt_lasts.append(ml)
            ind = pstat.tile([ROWS, 2], F32)   # per-sample indicator lhsT
            nc.vector.memset(ind, 0.0)
            nc.vector.memset(ind[0:NCH, 0:1], 1.0)
            nc.vector.tensor_scalar(
                out=ind[0:ROWS, 1:2], in0=ind[0:ROWS, 0:1],
                scalar1=-1.0, scalar2=1.0, op0=ALU.mult, op1=ALU.add,
            )

            # small-input DMAs on the gpsimd ring (idle during the stream;
            # the sync ring is dedicated to the mel tiles)
            specH = pstat.tile([ROWS, 261], F32)  # spec at t = 256p-3+h
            nc.vector.memset(specH, 0.0)
            r = pstat.tile([ROWS, 256], F32)
            nc.vector.memset(r, 0.0)

            def _dma_overlap(dst, src_1d, row_lo, row_hi, col_off, width, t_base):
                ap = bass.AP(
                    tensor=src_1d.tensor,
                    offset=src_1d.offset + t_base,
                    ap=[[256, row_hi - row_lo], [1, width]],
                )
                nc.gpsimd.dma_start(
                    out=dst[row_lo:row_hi, col_off : col_off + width], in_=ap
                )

            for b in range(BPC):
                r0 = b * NCH
                sp = spec[b]
                _dma_overlap(specH, sp, r0, r0 + 1, 3, 258, 0)
                # spec[0] into the t=-1 slot so d=0 -> spec_sim(t=0)=1
                _dma_overlap(specH, sp, r0, r0 + 1, 2, 1, 0)
                _dma_overlap(specH, sp, r0 + 1, r0 + 46, 0, 261, 256 - 3)
                _dma_overlap(specH, sp, r0 + 46, r0 + 47, 0, 227, 256 * 46 - 3)
                ini = init[b]
                _dma_overlap(r, ini, r0, r0 + 46, 0, 256, 0)
                _dma_overlap(r, ini, r0 + 46, r0 + 47, 0, LASTW, 256 * 46)
            validC = pstat.tile([ROWS, 260], F32)
            nc.gpsimd.dma_start(out=validC, in_=validC_d[:, :])
            validD = pstat.tile([ROWS, 256], F32)
            nc.gpsimd.dma_start(out=validD, in_=validD_d[:, :])
            scalemask = pstat.tile([ROWS, 256], F32)
            nc.gpsimd.dma_start(out=scalemask, in_=scalemask_d[:, :])
            ind2 = pstat.tile([2, ROWS], F32)
            nc.gpsimd.dma_start(out=ind2, in_=ind2_d[:, :])

            # ---------------- PSUM chunk-layout accumulators ----------------
            psS_ = pps.tile([128, 512], F32)
            psQ_ = pps.tile([128, 512], F32)
            psD_ = pps.tile([128, 512], F32)
            psS = psS_[:, 0:260]  # S: t = 256p - 2 + h
            psQ = psQ_[:, 0:262]  # Q: t = 256p - 3 + h (even N for f32r-era)
            psD = psD_[:, 0:260]  # D: t = 256p - 3 + h

            # ---------------- streaming + interleaved fill-in ----------------
            seen = [0, 0, 0]

            def emit_tile(b, j, g0, ng):
                t0 = g0 * CH
                wmel = ng * CH + 6
                if j == 0:
                    melt = melt_firsts[b]
                elif j == len(NGS) - 1:
                    melt = melt_lasts[b]
                else:
                    melt = pmel.tile([128, wmel], F32R, tag="melt")
                lo_pad = 3 if j == 0 else 0
                src_lo = t0 - 3 + lo_pad
                src_hi = min(T, t0 + ng * CH + 3)
                w_real = src_hi - src_lo
                nc.sync.dma_start(
                    out=melt[:, lo_pad : lo_pad + w_real],
                    in_=mel[b, :, src_lo:src_hi],
                )
                sq = psq.tile([128, wmel], BF16, tag="sq")
                wcr = ng * CH + 4
                cross = pcross.tile([128, wcr], BF16, tag="cross")
                if ng >= 4:
                    half = (ng // 2) * CH + 6  # covers chunk windows 0..ng/2-1
                    nc.scalar.activation(
                        out=sq[:, 0:half], in_=melt[:, 0:half], func=ACTF.Square
                    )
                    nc.vector.tensor_tensor(
                        out=cross[:, 0:half],
                        in0=melt[:, 0:half],
                        in1=melt[:, 1 : half + 1],
                        op=ALU.mult,
                    )
                    h0 = (ng // 2) * CH
                    nc.scalar.activation(
                        out=sq[:, h0:wmel], in_=melt[:, h0:wmel], func=ACTF.Square
                    )
                    nc.vector.tensor_tensor(
                        out=cross[:, h0:wcr],
                        in0=melt[:, h0:wcr],
                        in1=melt[:, h0 + 1 : wcr + 1],
                        op=ALU.mult,
                    )
                else:
                    nc.scalar.activation(out=sq, in_=melt, func=ACTF.Square)
                    nc.vector.tensor_tensor(
                        out=cross, in0=melt[:, 0:wcr], in1=melt[:, 1 : wcr + 1],
                        op=ALU.mult,
                    )
                for k in range(ng):
                    row = b * NCH + g0 + k
                    lhsT = WZ[:, 128 - row : 256 - row]
                    lhsTb = WZb[:, 128 - row : 256 - row]
                    c0 = k * CH
                    for ti, (pst, lh, rhs) in enumerate(
                        (
                            (psS, lhsT, melt[:, c0 + 1 : c0 + 261]),
                            (psQ, lhsTb, sq[:, c0 : c0 + 262]),
                            (psD, lhsTb, cross[:, c0 : c0 + 260]),
                        )
                    ):
                        seen[ti] += 1
                        nc.tensor.matmul(
                            out=pst[0:128, 0 : rhs.shape[1]],
                            lhsT=lh,
                            rhs=rhs,
                            start=(seen[ti] == 1),
                            stop=(seen[ti] == ROWS),
                        )

            def emit_spec_chain():
                # spec-sim chain; inputs ready early (gpsimd-ring DMAs)
                d = pstat.tile([ROWS, 260], F32)
                nc.vector.tensor_tensor(
                    out=d, in0=specH[:, 1:261], in1=specH[:, 0:260],
                    op=ALU.subtract,
                )
                ad = pstat.tile([ROWS, 260], F32)
                nc.scalar.activation(out=ad, in_=d, func=ACTF.Abs)
                a1 = pstat.tile([ROWS, 260], F32)
                nc.vector.tensor_scalar_add(out=a1, in0=ad, scalar1=1.0)
                srec = pstat.tile([ROWS, 260], F32)
                nc.vector.reciprocal_approx_fast(out=srec, in_=a1)
                t1s = pstat.tile([ROWS, 260], F32)
                nc.vector.tensor_scalar_mul(out=t1s, in0=srec, scalar1=float(w1))
                return t1s

            def emit_precompute():
                # refinement quantities that depend only on the initial r
                g05 = pstat.tile([ROWS, 256], F32)
                nc.vector.tensor_scalar(
                    out=g05, in0=r, scalar1=0.5, scalar2=None, op0=ALU.is_gt
                )
                rU = pstat.tile([ROWS, 256], F32)
                nc.vector.tensor_scalar(
                    out=rU, in0=r, scalar1=0.5, scalar2=1.0,
                    op0=ALU.add, op1=ALU.min,
                )
                rDA = pstat.tile([ROWS, 256], F32)
                nc.vector.tensor_scalar(
                    out=rDA, in0=r, scalar1=0.5, scalar2=0.0,
                    op0=ALU.subtract, op1=ALU.max,
                )
                # k = ceil(10*r-5) in (0..5] via compare ladder (ints exact)
                y = pstat.tile([ROWS, 256], F32)
                nc.vector.tensor_scalar(
                    out=y, in0=r, scalar1=10.0, scalar2=5.0,
                    op0=ALU.mult, op1=ALU.subtract,
                )
                cmps = []
                for jth in range(5):
                    c = pstat.tile([ROWS, 256], F32, tag=f"cmp{jth}")
                    nc.vector.tensor_scalar(
                        out=c, in0=y, scalar1=float(jth), scalar2=None,
                        op0=ALU.is_gt,
                    )
                    cmps.append(c)
                ka = pstat.tile([ROWS, 256], F32)
                nc.gpsimd.tensor_tensor(out=ka, in0=cmps[0], in1=cmps[1], op=ALU.add)
                kb = pstat.tile([ROWS, 256], F32)
                nc.gpsimd.tensor_tensor(out=kb, in0=cmps[2], in1=cmps[3], op=ALU.add)
                nc.gpsimd.tensor_tensor(out=ka, in0=ka, in1=kb, op=ALU.add)
                nc.gpsimd.tensor_tensor(out=ka, in0=ka, in1=cmps[4], op=ALU.add)
                rD0 = pstat.tile([ROWS, 256], F32)
                nc.vector.scalar_tensor_tensor(
                    out=rD0, in0=ka, scalar=-0.1, in1=r, op0=ALU.mult, op1=ALU.add
                )
                return g05, rU, rDA, rD0

            # sample 0, with the spec chain / precompute interleaved so the
            # DVE picks them up in stream gaps (their data is ready early)
            g0 = 0
            for j, ng in enumerate(NGS):
                emit_tile(0, j, g0, ng)
                emit_tile(1, j, g0, ng)
                g0 += ng
            # all tail inputs ready long ago; emitted after the stream so the
            # static DVE order keeps the stream dense
            t1s = emit_spec_chain()
            g05, rU, rDA, rD0 = emit_precompute()

            # ---------------- tail ----------------
            # --- temporal chain (independent of the cons chain) ---
            # W = 5-window sums of S (smooth = W*(0.2/128), folded into
            # scalars; S/128 is an exact exponent shift)
            W = pstat.tile([ROWS, 256], F32)
            psS_ap = psS_[0:ROWS, 0:1]
            win5 = bass.AP(
                tensor=psS_ap.tensor, offset=psS_ap.offset,
                ap=[[512, ROWS], [1, 256], [1, 5]],
            )
            nc.vector.tensor_reduce(out=W, in_=win5, axis=AX.X, op=ALU.add)
            Wsq = pstat.tile([ROWS, 256], F32)
            nc.scalar.activation(out=Wsq, in_=W, func=ACTF.Square)
            psT1 = pps2.tile([2, 256], F32)
            psT2 = pps2.tile([2, 256], F32)
            nc.tensor.matmul(
                out=psT1, lhsT=ind[0:ROWS, 0:2], rhs=W[0:ROWS, :],
                start=True, stop=True,
            )
            nc.tensor.matmul(
                out=psT2, lhsT=ind[0:ROWS, 0:2], rhs=Wsq[0:ROWS, :],
                start=True, stop=True,
            )
            SMSC = 0.2 / 128.0
            sx = pstat.tile([2, 1], F32)
            sxx = pstat.tile([2, 1], F32)
            nc.vector.tensor_reduce(out=sx, in_=psT1, axis=AX.X, op=ALU.add)
            nc.vector.tensor_reduce(out=sxx, in_=psT2, axis=AX.X, op=ALU.add)
            nc.vector.tensor_scalar_mul(out=sx, in0=sx, scalar1=float(SMSC))
            nc.vector.tensor_scalar_mul(out=sxx, in0=sxx, scalar1=float(SMSC * SMSC))
            s2 = pstat.tile([2, 1], F32)
            nc.vector.tensor_tensor(out=s2, in0=sx, in1=sx, op=ALU.mult)
            s3 = pstat.tile([2, 1], F32)
            nc.vector.tensor_scalar_mul(out=s3, in0=s2, scalar1=1.0 / float(T))
            var = pstat.tile([2, 1], F32)
            nc.vector.tensor_tensor(out=var, in0=sxx, in1=s3, op=ALU.subtract)
            nc.vector.tensor_scalar_mul(out=var, in0=var, scalar1=1.0 / float(T - 1))
            std = pstat.tile([2, 1], F32)
            nc.scalar.activation(out=std, in_=var, func=ACTF.Sqrt)
            # w2t = w2*(1-std); thresholds thrH = 0.7 - w2t, thrL = 0.4 - w2t
            w2t2 = pstat.tile([2, 1], F32)
            nc.vector.tensor_scalar(
                out=w2t2, in0=std, scalar1=-1.0, scalar2=1.0,
                op0=ALU.mult, op1=ALU.add,
            )
            nc.vector.tensor_scalar_mul(out=w2t2, in0=w2t2, scalar1=float(w2))
            thr2 = pstat.tile([2, 2], F32)
            nc.vector.tensor_scalar(
                out=thr2[0:2, 0:1], in0=w2t2, scalar1=-1.0, scalar2=0.7,
                op0=ALU.mult, op1=ALU.add,
            )
            nc.vector.tensor_scalar(
                out=thr2[0:2, 1:2], in0=w2t2, scalar1=-1.0, scalar2=0.4,
                op0=ALU.mult, op1=ALU.add,
            )
            psB = pps2.tile([ROWS, 2], F32)
            nc.tensor.matmul(
                out=psB, lhsT=ind2[0:2, 0:ROWS], rhs=thr2[0:2, 0:2],
                start=True, stop=True,
            )
            thr = pstat.tile([ROWS, 2], F32)
            nc.scalar.activation(out=thr, in_=psB, func=ACTF.Copy)

            # --- cons chain ---
            nmH = pstat.tile([ROWS, 261], F32)
            nc.scalar.activation(out=nmH, in_=psQ[0:ROWS, 0:261], func=ACTF.Sqrt)
            nc.vector.tensor_scalar_max(out=nmH, in0=nmH, scalar1=float(EPS))
            den = pstat.tile([ROWS, 260], F32)
            nc.vector.tensor_tensor(
                out=den, in0=nmH[:, 0:260], in1=nmH[:, 1:261], op=ALU.mult
            )
            rec = pstat.tile([ROWS, 260], F32)
            nc.vector.reciprocal_approx_fast(out=rec, in_=den)
            cosH = pstat.tile([ROWS, 260], F32)
            nc.vector.tensor_tensor(out=cosH, in0=psD[0:ROWS, :], in1=rec, op=ALU.mult)
            # cons0 = w0*mel_sim + w1*spec_sim (mel_sim[t] = cos[t-1] = same h)
            consH = pstat.tile([ROWS, 260], F32)
            nc.vector.scalar_tensor_tensor(
                out=consH, in0=cosH, scalar=float(w0), in1=t1s,
                op0=ALU.mult, op1=ALU.add,
            )
            nc.vector.tensor_tensor(out=consH, in0=consH, in1=validC, op=ALU.mult)

            # local means via sliding-window reduce; scalemask has the /4 edges
            LS = pstat.tile([ROWS, 256], F32)
            ch_ap = consH[0:ROWS, 0:1]
            win5c = bass.AP(
                tensor=ch_ap.tensor, offset=ch_ap.offset,
                ap=[[260, ROWS], [1, 256], [1, 5]],
            )
            nc.vector.tensor_reduce(out=LS, in_=win5c, axis=AX.X, op=ALU.add)
            local = pstat.tile([ROWS, 256], F32)
            nc.vector.tensor_tensor(out=local, in0=LS, in1=scalemask, op=ALU.mult)

            # grads: A = (g*g > thresh^2)  (avoids an ACT Abs table switch)
            g = pstat.tile([ROWS, 256], F32)
            nc.vector.tensor_tensor(
                out=g, in0=consH[:, 2:258], in1=consH[:, 1:257], op=ALU.subtract
            )
            gsq = pstat.tile([ROWS, 256], F32)
            nc.vector.tensor_tensor(out=gsq, in0=g, in1=g, op=ALU.mult)
            A = pstat.tile([ROWS, 256], F32)
            th2 = float(np.float32(GRAD_THRESH) * np.float32(GRAD_THRESH))
            nc.vector.tensor_scalar(
                out=A, in0=gsq, scalar1=th2, scalar2=None, op0=ALU.is_gt
            )

            # ddir = (local<thrL) - (local>thrH), times 0.1*validD
            u = pstat.tile([ROWS, 256], F32)
            nc.vector.tensor_scalar(
                out=u, in0=local, scalar1=thr[:, 0:1], scalar2=None, op0=ALU.is_gt
            )
            v = pstat.tile([ROWS, 256], F32)
            nc.vector.tensor_scalar(
                out=v, in0=local, scalar1=thr[:, 1:2], scalar2=None, op0=ALU.is_lt
            )
            ddir = pstat.tile([ROWS, 256], F32)
            nc.vector.tensor_tensor(out=ddir, in0=v, in1=u, op=ALU.subtract)
            nc.vector.tensor_tensor(out=ddir, in0=ddir, in1=validD, op=ALU.mult)

            # --- closed-form refinement combine ---
            up = pstat.tile([ROWS, 256], F32)
            nc.vector.tensor_scalar(
                out=up, in0=ddir, scalar1=0.0, scalar2=None, op0=ALU.is_gt
            )
            dn = pstat.tile([ROWS, 256], F32)
            nc.vector.tensor_scalar(
                out=dn, in0=ddir, scalar1=0.0, scalar2=None, op0=ALU.is_lt
            )
            act0 = pstat.tile([ROWS, 256], F32)
            nc.vector.tensor_tensor(out=act0, in0=g05, in1=A, op=ALU.max)
            nA = pstat.tile([ROWS, 256], F32)
            nc.vector.tensor_scalar(
                out=nA, in0=A, scalar1=-1.0, scalar2=1.0, op0=ALU.mult, op1=ALU.add
            )
            mU = pstat.tile([ROWS, 256], F32)
            nc.vector.tensor_tensor(out=mU, in0=up, in1=act0, op=ALU.mult)
            mDA = pstat.tile([ROWS, 256], F32)
            nc.vector.tensor_tensor(out=mDA, in0=dn, in1=A, op=ALU.mult)
            mD0 = pstat.tile([ROWS, 256], F32)
            nc.vector.tensor_tensor(out=mD0, in0=dn, in1=nA, op=ALU.mult)
            nc.vector.tensor_tensor(out=mD0, in0=mD0, in1=g05, op=ALU.mult)
            nc.vector.copy_predicated(
                out=r, mask=mU.bitcast(mybir.dt.int32), data=rU
            )
            nc.vector.copy_predicated(
                out=r, mask=mDA.bitcast(mybir.dt.int32), data=rDA
            )
            nc.vector.copy_predicated(
                out=r, mask=mD0.bitcast(mybir.dt.int32), data=rD0
            )
            nc.vector.tensor_scalar(
                out=r, in0=r, scalar1=0.0, scalar2=1.0, op0=ALU.max, op1=ALU.min
            )

            # ---------------- output (two DMA rings) ----------------
            for b in range(BPC):
                r0 = b * NCH
                ob = out[b]
                eng = nc.sync if b == 0 else nc.scalar
                eng.dma_start(
                    out=bass.AP(
                        tensor=ob.tensor, offset=ob.offset, ap=[[256, 46], [1, 256]]
                    ),
                    in_=r[r0 : r0 + 46, :],
                )
                eng.dma_start(
                    out=bass.AP(
                        tensor=ob.tensor,
                        offset=ob.offset + 256 * 46,
                        ap=[[256, 1], [1, LASTW]],
                    ),
                    in_=r[r0 + 46 : r0 + 47, 0:LASTW],
                )

    nc.compile()
    return nc


_CACHE = {}


def _get_nc(wbytes):
    if wbytes not in _CACHE:
        w = np.frombuffer(wbytes, np.float32)
        _CACHE[wbytes] = build_nc(float(w[0]), float(w[1]), float(w[2]))
    return _CACHE[wbytes]


def kernel(**inputs):
    mel = np.ascontiguousarray(np.asarray(inputs["mel_features"], np.float32))
    spec = np.ascontiguousarray(np.asarray(inputs["spectral_features"], np.float32))
    init = np.ascontiguousarray(np.asarray(inputs["initial_boundaries"], np.float32))
    sw = np.asarray(inputs["similarity_weights"], np.float32)
    w = _softmax_f32(sw)
    nc = _get_nc(w.tobytes())

    in_maps = []
    for c in range(NCORES):
        s = slice(c * BPC, (c + 1) * BPC)
        in_maps.append(
            {
                "mel_features": np.ascontiguousarray(mel[s]),
                "spectral_features": np.ascontiguousarray(spec[s]),
                "initial_boundaries": np.ascontiguousarray(init[s]),
            }
        )
    res = run_bass_kernel_spmd(nc, in_maps, core_ids=list(range(NCORES)))
    global _LAST_RESULT
    _LAST_RESULT = res
    outs = [np.asarray(res.results[c]["out"], np.float32) for c in range(NCORES)]
    return np.concatenate(outs, axis=0)


_LAST_RESULT = None


if __name__ == "__main__":
    nc = build_nc(1 / 3, 1 / 3, 1 / 3)
    ninst = sum(len(b.instructions) for b in nc.m.functions[0].blocks)
    print("built ok, instructions:", ninst)

